# revision 23
# baseline (speedup 1.0000x reference)
"""Trainium2 Bass kernel for nn_Attention_197568495719.

Full attention layer: QKV projection + RoPE + int8 KV quant-dequant + GQA
causal SDPA + output projection.  B=2, S=2048, D=2048, 16 q heads / 4 kv
heads, head_dim=128.

Sharding: 8 cores = 2 (batch) x 4 (kv-head groups).  Core (b, g) computes
batch b with q heads 4g..4g+3 and kv head g (tensor parallel on heads:
wq/wk/wv split on output dim, wo on input dim).  Each core produces a
partial outT = (attn @ wo_g).T in [D, S] layout; the host sums the 4
group partials per batch and transposes back.

Design (measured ~290us full-clock vs the 485us v1 baseline; the
device times bimodally, ~1.18x slower when power-throttled to ~2GHz):
- Phase 1 (projections): 512-token chunks at N=512 moving; data/wq/wkv
  fed as bf16 (same PE rate as fp32r, half the DMA bytes + SBUF).  k/v
  are projected directly into [token, dim] tiles so the int8 quant path
  needs no PE transposes in; k RoPE runs along the free axis with a
  sign-folded sin table.  All quant-dequant work rides inside phase 1
  (per 512-token group as soon as its k is roped) so the attention phase
  has no serial DVE chains.  q RoPE is applied in place per (head,
  chunk); rot matmuls are deferred behind all four head projections so
  their input copies are long done when the PE reaches them.  A scratch
  fp32 matmul burst warms the PE HAM clock-gate during the initial DMAs.
- Phase 2 (attention): two head-streams interleaved per chunk with
  scores emitted four ki ahead of the accumulating matmuls, so the PE
  always has independent work while ACT exps complete.  Causal masking
  multiplies only the 128x128 triangular block per diagonal tile (on the
  otherwise-idle GPSIMD); exp is trimmed to the live q-range.  Softmax
  accumulators are staged out of PSUM with cheap DVE copies at pair end
  (keeping all 8 PSUM banks rotating; on ACT these copies would queue
  ahead of the next pair's exps and stall score-slot recycling), and 1/sum is computed per pair as
  exp(-ln(sum)) on ACT -- Ln and Exp share an ACT function table, so no
  ACT_TABLE_LOAD ever splits the exp stream (Reciprocal lives in a
  different table; each switch costs 1.28us of serial ACT time).  Each
  pair's finalize is deferred into the middle of the next pair.  The
  previous chunk's output-projection units are interleaved into pair-2's
  ki loop as PE filler; the final output projection is software-pipelined
  7 deep across all 8 PSUM banks (attention accumulators are dead by
  then) with a per-stream split of the last softmax finalize, and its
  store DMAs
  fan out across all three DMA-capable queues.  wo/attn tiles and the
  outT partials are bf16, halving the weight-load and output-store HBM
  traffic shared by all 8 cores (host accumulates partials in fp32).
- All PE matmuls are float32r (full-rate fp32 at moving dim >= 256)
  except the bf16 projections; rounding in the quant path uses the fp32
  +-1.5*2^23 magic-add trick (exact round-half-to-even, matching
  jnp.round).
"""

import numpy as np

import bass_rust
import concourse.bass as bass
import concourse.tile as tile
import concourse.mybir as mybir
from concourse.bass_utils import run_bass_kernel_spmd

B, S, D = 2, 2048, 2048
NH, NKV, HD = 16, 4, 128
GQ = 512            # q dims per core (4 heads)
NKO = D // 128      # 16 contraction tiles
PC = 512            # projection/attention chunk width (tokens)
NPC = S // PC       # 4
QC = 512
NQC = S // QC       # 4
MAGIC = float(np.float32(12582912.0))  # 1.5 * 2**23
SM_SCALE = 1.0 / float(np.sqrt(HD))

F32 = mybir.dt.float32
F32R = mybir.dt.float32r
BF16 = mybir.dt.bfloat16
MULT = mybir.AluOpType.mult
ADD = mybir.AluOpType.add
EXP = mybir.ActivationFunctionType.Exp

_CACHE = {}

# retained after each kernel() call so test harnesses can read profiling info
LAST_RESULTS = None


def _split_multi_waits(nc):
    """This walrus build caps sync waits at 1 per instruction.  Hoist extra
    waits onto single-wait NoOps immediately preceding the instruction on
    the same engine (identical semantics: the engine is in-order)."""
    for f in nc.m.functions:
        for bb in f.blocks:
            new = []
            for inst in bb.instructions:
                si = inst.sync_info
                if si is None:
                    new.append(inst)
                    continue
                waits = list(si.on_wait)
                if len(waits) > 1:
                    for k, w in enumerate(waits[:-1]):
                        nop = mybir.InstNoOp(name=f"{inst.name}-w{k}", ins=[], outs=[])
                        nop.engine = inst.engine
                        nop.sync_info = bass_rust.SyncInfo(on_wait=[w], on_update=[])
                        new.append(nop)
                    inst.sync_info = bass_rust.SyncInfo(
                        on_wait=[waits[-1]], on_update=list(si.on_update)
                    )
                new.append(inst)
            bb.instructions = new


def _host_consts():
    theta = 10000.0
    angles = 1.0 / theta ** (np.arange(0, HD, 2, dtype=np.float32) / HD)
    emb = np.outer(np.arange(S, dtype=np.float32), angles)
    emb = np.concatenate([emb, emb], axis=-1)          # [S, HD]
    cos = np.cos(emb).astype(np.float32)               # [S, HD]
    sin = np.sin(emb).astype(np.float32)
    cosT = np.ascontiguousarray(cos.T)                 # [128, S]
    sinT = np.ascontiguousarray(sin.T)

    # [t, d]-layout tables for k rope: [p, t_tile, hd]
    ctd = np.ascontiguousarray(cos.reshape(S // 128, 128, HD).transpose(1, 0, 2))
    std = sin.reshape(S // 128, 128, HD).transpose(1, 0, 2).copy()
    sgn = std.copy()
    sgn[:, :, : HD // 2] = -std[:, :, : HD // 2]       # sign-folded sin
    sgn = np.ascontiguousarray(sgn)

    rot = np.zeros((128, 128), dtype=np.float32)       # lhsT of rotate_half
    for i in range(64):
        rot[i, i + 64] = 1.0
        rot[i + 64, i] = -1.0

    p = np.arange(128)[:, None]
    f = np.arange(128)[None, :]
    tril = (p <= f).astype(np.float32)                 # key p visible to q f

    ones = np.ones((128, 128), dtype=np.float32)
    ident = np.eye(128, dtype=np.float32)
    import ml_dtypes
    bf16 = ml_dtypes.bfloat16
    return {
        "cosT": cosT.astype(bf16), "sinT": sinT.astype(bf16),
        "ctd": ctd.astype(bf16), "sgn": sgn.astype(bf16),
        "rot": rot.astype(bf16), "tril": tril.astype(bf16),
        "ones": ones, "ident": ident,
    }


def _build_nc():
    nc = bass.Bass("TRN2", target_bir_lowering=False, debug=False)

    # host pre-arranges dataT/wq/wkv into partition-major layouts so every
    # DMA element is >=4KB contiguous (512B elements run ~3x slower)
    dataT = nc.dram_tensor("dataT", [128, NPC, NKO, PC], BF16,
                           kind="ExternalInput").ap()
    wq = nc.dram_tensor("wq", [128, NKO, GQ], BF16, kind="ExternalInput").ap()
    wkv = nc.dram_tensor("wkv", [128, NKO, 2 * HD], BF16,
                         kind="ExternalInput").ap()
    wo = nc.dram_tensor("wo", [GQ, D], BF16, kind="ExternalInput").ap()
    cosT_d = nc.dram_tensor("cosT", [128, S], BF16, kind="ExternalInput").ap()
    sinT_d = nc.dram_tensor("sinT", [128, S], BF16, kind="ExternalInput").ap()
    ctd_d = nc.dram_tensor("ctd", [128, NKO, HD], BF16, kind="ExternalInput").ap()
    sgn_d = nc.dram_tensor("sgn", [128, NKO, HD], BF16, kind="ExternalInput").ap()
    rot_d = nc.dram_tensor("rot", [128, 128], BF16, kind="ExternalInput").ap()
    tril_d = nc.dram_tensor("tril", [128, 128], BF16, kind="ExternalInput").ap()
    ones_d = nc.dram_tensor("ones", [128, 128], F32R, kind="ExternalInput").ap()
    ident_d = nc.dram_tensor("ident", [128, 128], F32R, kind="ExternalInput").ap()
    outT = nc.dram_tensor("outT", [D, S], BF16, kind="ExternalOutput").ap()

    dataT_r = dataT                                          # [128, 4, 16, PC]
    wq_r = wq                                                # [128, 16, 512]
    wkv_r = wkv                                              # [128, 16, 256]
    wo_r = wo.rearrange("(h p) n -> p h n", p=128)           # [128, 4, S]
    outT_p = outT.rearrange("(dt p) t -> p dt t", p=128)     # [128, 16, S]

    from contextlib import ExitStack
    with tile.TileContext(nc) as tc, ExitStack() as stack:
        small_consts = stack.enter_context(tc.tile_pool(name="sconsts", bufs=1))
        rot_sb = small_consts.tile([128, 128], BF16)
        ones_sb = small_consts.tile([128, 128], F32R)
        id_sb = small_consts.tile([128, 128], F32R)
        tril_sb = small_consts.tile([128, 128], BF16)

        persist = stack.enter_context(tc.tile_pool(name="persist", bufs=1))
        xq4 = persist.tile([128, 4, S], BF16, name="xq4")    # roped q, [d, h, t]
        kt4 = persist.tile([128, 4, QC], BF16, name="kt4")   # quant k, [d, g, t]
        v_g = [persist.tile([128, 4, HD], BF16, tag=f"vg{g}", name=f"v_g{g}")
               for g in range(4)]                            # quant v, [t, j, d]

        GRP = 4

        # ---------------- Phase 1: projections + rope + quant ----------------
        with tc.tile_pool(name="p1consts", bufs=1) as p1c, \
             tc.tile_pool(name="wpool", bufs=1) as wpool, \
             tc.tile_pool(name="datapool", bufs=2) as datapool, \
             tc.tile_pool(name="kvstage", bufs=2) as kvstage, \
             tc.tile_pool(name="qtmp", bufs=2) as qtmp, \
             tc.tile_pool(name="t2pool", bufs=3) as t2pool, \
             tc.tile_pool(name="proj_ps", bufs=3, space="PSUM") as proj_ps, \
             tc.tile_pool(name="kv_ps", bufs=2, space="PSUM") as kv_ps, \
             tc.tile_pool(name="rope_ps", bufs=2, space="PSUM") as rope_ps, \
             tc.tile_pool(name="tp_ps", bufs=1, space="PSUM") as tp_ps:
            cos_sb = p1c.tile([128, S], BF16)
            sin_sb = p1c.tile([128, S], BF16)
            ctd_sb = p1c.tile([128, NKO, HD], BF16)
            sgn_sb = p1c.tile([128, NKO, HD], BF16)
            wq_sb = wpool.tile([128, NKO, GQ], BF16)
            wkv_sb = wpool.tile([128, NKO, 2 * HD], BF16)

            dT = {}
            for c in range(2):
                dT[c] = datapool.tile([128, NKO, PC], BF16, tag="dT",
                                      name=f"dT{c}")

            # initial loads across all 3 DMA-capable queues (sync/gpsimd/
            # scalar).  The DMA arbiter round-robins PACKETS across queues,
            # so a queue carrying small-element descriptors gets starved:
            # big 4-8KB-element transfers go first in each queue's FIFO,
            # ordered by first use (kv0: wkv+dT0, q0: wq, kv1: dT1), and the
            # small consts ride at the queue tails.
            nc.sync.dma_start(dT[0][:, 0:8], dataT_r[:, 0, 0:8])
            nc.gpsimd.dma_start(dT[0][:, 8:16], dataT_r[:, 0, 8:16])
            nc.scalar.dma_start(wkv_sb[:, 0:8], wkv_r[:, 0:8])
            nc.scalar.dma_start(wkv_sb[:, 8:16], wkv_r[:, 8:16])
            nc.sync.dma_start(wq_sb[:, 0:8], wq_r[:, 0:8])
            nc.scalar.dma_start(wq_sb[:, 8:16], wq_r[:, 8:16])
            nc.gpsimd.dma_start(ctd_sb[:], ctd_d[:])
            nc.gpsimd.dma_start(cos_sb[:], cosT_d[:])
            nc.sync.dma_start(dT[1][:, 0:8], dataT_r[:, 1, 0:8])
            nc.gpsimd.dma_start(dT[1][:, 8:16], dataT_r[:, 1, 8:16])
            nc.scalar.dma_start(sgn_sb[:], sgn_d[:])
            nc.scalar.dma_start(sin_sb[:], sinT_d[:])
            nc.sync.dma_start(rot_sb[:], rot_d[:])
            nc.sync.dma_start(id_sb[:], ident_d[:])
            nc.sync.dma_start(ones_sb[:], ones_d[:])
            nc.sync.dma_start(tril_sb[:], tril_d[:])

            # PE warm-up/filler: scratch matmuls during the initial DMA wait
            # keep the HAM clock-gate at K=8/8 so real work runs at full
            # clock, and bridge to the kv projection's first data (~17us in)
            # so no >3.4us idle window re-throttles the clock.
            warm = wpool.tile([128, QC], BF16, name="warm_scratch")
            nc.vector.memset(warm[:], 0.0)

            def warm_fill(n, w=QC):
                for _ in range(n):
                    wps = rope_ps.tile([128, QC], F32, tag="pr")
                    nc.tensor.matmul(wps[:, 0:w], warm[:, 0:128], warm[:, 0:w],
                                     start=True, stop=True)

            warm_fill(22)

            def quant_group(src_ap, dst_ap):
                amax = qtmp.tile([128, GRP, 1], F32, tag="amax")
                scl = qtmp.tile([128, GRP, 1], F32, tag="scl")
                inv = qtmp.tile([128, GRP, 1], F32, tag="inv")
                xs = qtmp.tile([128, GRP, HD], F32, tag="xs")
                nc.vector.tensor_reduce(amax[:], src_ap, mybir.AxisListType.X,
                                        mybir.AluOpType.max,
                                        apply_absolute_value=True)
                nc.vector.tensor_scalar_max(amax[:], amax[:], 1e-8)
                nc.vector.tensor_scalar_mul(scl[:], amax[:], 1.0 / 127.0)
                nc.vector.reciprocal(inv[:], scl[:])
                sclb = scl[:].to_broadcast((128, GRP, HD))
                invb = inv[:].to_broadcast((128, GRP, HD))
                nc.vector.tensor_tensor(xs[:], src_ap, invb, MULT)
                nc.vector.tensor_scalar_add(xs[:], xs[:], MAGIC)
                nc.vector.tensor_scalar_add(xs[:], xs[:], -MAGIC)
                nc.vector.tensor_tensor(dst_ap, xs[:], sclb, MULT)

            for c in range(NPC):
                csl = bass.ts(c, PC)
                if c + 2 < NPC:
                    cb = c + 2
                    t_ = datapool.tile([128, NKO, PC], BF16, tag="dT",
                                       name=f"dT{cb}")
                    dT[cb] = t_
                    eng = nc.gpsimd if cb % 2 else nc.sync
                    eng.dma_start(t_[:, 0:8], dataT_r[:, cb, 0:8])
                    eng.dma_start(t_[:, 8:16], dataT_r[:, cb, 8:16])

                # --- k/v projection straight into [t, d] tiles ---
                kv_td = kvstage.tile([128, GRP, 2 * HD], F32, tag="kvtd",
                                     name=f"kvtd{c}")
                for j in range(GRP):
                    pkv = kv_ps.tile([128, 2 * HD], F32, tag="pkv")
                    for ko in range(NKO):
                        nc.tensor.matmul(pkv[:],
                                         dT[c][:, ko, bass.ds(j * 128, 128)],
                                         wkv_sb[:, ko],
                                         start=(ko == 0), stop=(ko == NKO - 1))
                    nc.scalar.copy(kv_td[:, j, :], pkv[:])

                # --- k rope along free axis (sign-folded sin table) ---
                kr = kvstage.tile([128, GRP, HD], F32, tag="kr", name=f"kr{c}")
                t2k = qtmp.tile([128, GRP, HD], F32, tag="t2k")
                tsl = bass.ts(c, GRP)  # 4 token tiles of this group
                nc.vector.tensor_tensor(kr[:], kv_td[:, :, 0:HD],
                                        ctd_sb[:, tsl], MULT)
                nc.vector.tensor_tensor(t2k[:, :, 0:64],
                                        kv_td[:, :, 64:HD],
                                        sgn_sb[:, tsl, 0:64], MULT)
                nc.vector.tensor_tensor(t2k[:, :, 64:HD],
                                        kv_td[:, :, 0:64],
                                        sgn_sb[:, tsl, 64:HD], MULT)
                nc.vector.tensor_tensor(kr[:], kr[:], t2k[:], ADD)

                # --- int8 quant-dequant (k roped, v raw) ---
                kq = kvstage.tile([128, GRP, HD], F32R, tag="kq", name=f"kq{c}")
                quant_group(kr[:], kq[:])

                # --- q projection per head; kq transposes slotted after
                # h1 so the DVE quant chain has time to finish; rot matmuls
                # deferred behind all projections so their input copies are
                # long done when the PE reaches them ---
                def emit_qproj(h):
                    pq = proj_ps.tile([128, QC], F32, tag="pq",
                                      name=f"pq{c}_{h}")
                    for ko in range(NKO):
                        nc.tensor.matmul(pq[:], wq_sb[:, ko, bass.ts(h, 128)],
                                         dT[c][:, ko],
                                         start=(ko == 0), stop=(ko == NKO - 1))
                    nc.scalar.copy(xq4[:, h, csl], pq[:])

                emit_qproj(0)
                emit_qproj(1)
                for j in range(GRP):
                    pt = tp_ps.tile([128, 128], F32R, tag="tp")
                    nc.tensor.transpose(pt[:], kq[:, j, :], id_sb[:])
                    nc.scalar.copy(kt4[:, c, bass.ts(j, 128)], pt[:])
                emit_qproj(2)
                emit_qproj(3)
                for h in range(4):
                    pr = rope_ps.tile([128, QC], F32, tag="pr")
                    nc.tensor.matmul(pr[:], rot_sb[:], xq4[:, h, csl],
                                     start=True, stop=True)
                    t1 = t2pool.tile([128, QC], BF16, tag="t1")
                    t2 = t2pool.tile([128, QC], BF16, tag="t2")
                    nc.vector.tensor_tensor(t1[:], xq4[:, h, csl],
                                            cos_sb[:, csl], MULT)
                    nc.vector.tensor_tensor(t2[:], pr[:], sin_sb[:, csl], MULT)
                    nc.vector.tensor_tensor(xq4[:, h, csl], t1[:], t2[:], ADD)
                # v_g[c] is not needed until attention chunk c; deferring
                # its DVE chain past the q ropes lets the rope PSUM banks
                # (reused by the next chunk's rot matmuls, and by phase-2
                # accumulator pools after the last chunk) free earlier
                quant_group(kv_td[:, :, HD:], v_g[c][:])

        # ---------------- Phase 2: attention + output projection ----------------
        # ki tiles are processed in units of 2 with [128,2,QC] "wide" tiles
        # spanning 2 PSUM banks / 2KB-per-partition SBUF spans: one exp per
        # off-diagonal unit (amortizes the 352-cycle ACT pipeline fill), one
        # staging copy / store per unit.  The softmax denominator is
        # accumulated in SBUF by DVE (stream 1) / GPSIMD (stream 0) adds as
        # each exp tile lands, so the PE runs one ones-matmul per stream per
        # pair instead of one per ki tile.
        with tc.tile_pool(name="p2", bufs=1) as p2, \
             tc.tile_pool(name="attn_sb", bufs=5) as attn_sb, \
             tc.tile_pool(name="exp_pool", bufs=7) as exp_pool, \
             tc.tile_pool(name="araw", bufs=3) as araw_pool, \
             tc.tile_pool(name="accp", bufs=4) as acc_pool, \
             tc.tile_pool(name="rc4p", bufs=2) as rc_pool, \
             tc.tile_pool(name="outstage", bufs=4) as outstage, \
             tc.tile_pool(name="score_ps", bufs=2, space="PSUM") as score_ps, \
             tc.tile_pool(name="attn_ps", bufs=1, space="PSUM") as attn_ps, \
             tc.tile_pool(name="fill_ps", bufs=1, space="PSUM") as fill_ps:
            wo_t = [p2.tile([128, S], BF16, tag=f"wo{h}", name=f"wo{h}")
                    for h in range(4)]
            for h in range(4):
                nc.sync.dma_start(wo_t[h][:], wo_r[:, h])

            def emit_out_unit(c_prev, tiles, pu, pool, tag, ceng, seng):
                # one output-projection unit: dt tiles (2pu, 2pu+1) of the
                # [D, QC] chunk column block -> 8 matmuls, 1 wide cast,
                # 1 wide store
                cpsl = bass.ts(c_prev, QC)
                po2 = pool.tile([128, 2, QC], F32, tag=tag)
                for half in range(2):
                    dt_ = 2 * pu + half
                    for h2 in range(4):
                        at2, sti = tiles[h2]
                        nc.tensor.matmul(po2[:, half],
                                         wo_t[h2][:, bass.ts(dt_, 128)],
                                         at2[:, sti],
                                         start=(h2 == 0), stop=(h2 == 3))
                ot2 = outstage.tile([128, 2, QC], BF16, tag="ot")
                if ceng is nc.vector:
                    ceng.tensor_copy(ot2[:], po2[:])
                else:
                    ceng.copy(ot2[:], po2[:])
                seng.dma_start(outT_p[:, 2 * pu:2 * pu + 2, cpsl], ot2[:])

            def out_proj_tail(c_prev, tiles):
                # kernel-tail: software-pipelined 3 units deep over the 4
                # wide PSUM slots; h0/h1 matmuls (streams finalized in the
                # previous pair) lead the h2/h3+cast+store passes, covering
                # the last pair's softmax finalize chain
                pools = [(score_ps, "ps2"), (score_ps, "ps2"),
                         (attn_ps, "pa2"), (fill_ps, "fill")]
                pos = {}

                def finishp(pu):
                    po2 = pos.pop(pu)
                    for half in range(2):
                        dt_ = 2 * pu + half
                        for h2 in (2, 3):
                            at2, sti = tiles[h2]
                            nc.tensor.matmul(po2[:, half],
                                             wo_t[h2][:, bass.ts(dt_, 128)],
                                             at2[:, sti],
                                             start=False, stop=(h2 == 3))
                    ot2 = outstage.tile([128, 2, QC], BF16, tag="ot")
                    if pu % 2:
                        nc.vector.tensor_copy(ot2[:], po2[:])
                    else:
                        nc.scalar.copy(ot2[:], po2[:])
                    eng = nc.gpsimd if pu % 2 else nc.sync
                    eng.dma_start(outT_p[:, 2 * pu:2 * pu + 2,
                                         bass.ts(c_prev, QC)], ot2[:])

                for pu in range(NKO // 2):
                    pool, tag = pools[pu % 4]
                    po2 = pool.tile([128, 2, QC], F32, tag=tag)
                    pos[pu] = po2
                    for half in range(2):
                        dt_ = 2 * pu + half
                        for h2 in (0, 1):
                            at2, sti = tiles[h2]
                            nc.tensor.matmul(po2[:, half],
                                             wo_t[h2][:, bass.ts(dt_, 128)],
                                             at2[:, sti],
                                             start=(h2 == 0), stop=False)
                    if pu >= 3:
                        finishp(pu - 3)
                for pu in range(NKO // 2 - 3, NKO // 2):
                    finishp(pu)

            prev = None
            LAG = 2  # units the score/exp pipeline leads the pa matmuls by

            def emit_pair(c, hA, hB, attn_tiles, carry_in, fillers=()):
                fillers = list(fillers)
                nki = 4 * (c + 1)
                U = nki // 2
                streams = (hA, hB)
                if carry_in is not None:
                    # previous pair's Ln runs first so its score-pool slot
                    # frees before this pair's second stream needs it
                    carry_in[0]()
                pa2 = attn_ps.tile([128, 2, QC], F32, tag="pa2",
                                   name=f"pa2_{c}_{hA}")
                acc = [acc_pool.tile([128, QC], F32R, tag="acc",
                                     name=f"acc{c}_{h}") for h in streams]

                def emit_pa(u, et2s, qo):
                    for st in range(2):
                        for half in range(2):
                            ki = 2 * u + half
                            q = qo[half]
                            nc.tensor.matmul(
                                pa2[:, st, q:], v_g[ki // 4][:, ki % 4],
                                et2s[st][:, half, q:],
                                start=(ki == 0), stop=(ki == nki - 1))

                pending = []
                for u in range(U):
                    k0 = 2 * u
                    diag = k0 >= 4 * c
                    qo = (128 * (k0 - 4 * c), 128 * (k0 + 1 - 4 * c)) \
                        if diag else (0, 0)
                    et2s = []
                    for st in range(2):
                        h = streams[st]
                        ps2 = score_ps.tile([128, 2, QC], F32, tag="ps2")
                        for half in range(2):
                            q = qo[half]
                            nc.tensor.matmul(
                                ps2[:, half, q:],
                                kt4[:, (k0 + half) // 4,
                                    bass.ts((k0 + half) % 4, 128)],
                                xq4[:, h, bass.ds(c * QC + q, QC - q)],
                                start=True, stop=True)
                        et2 = exp_pool.tile([128, 2, QC], BF16, tag="et2")
                        et2s.append(et2)
                        if diag:
                            # per-half exps: the trimmed halves start at
                            # different q offsets, and the garbage gap is
                            # unwritten PSUM
                            for half in range(2):
                                q = qo[half]
                                nc.scalar.activation(et2[:, half, q:],
                                                     ps2[:, half, q:], EXP,
                                                     scale=SM_SCALE)
                        else:
                            nc.scalar.activation(et2[:], ps2[:], EXP,
                                                 scale=SM_SCALE)
                        # causal mask + denominator accumulation; stream 0
                        # rides GPSIMD, stream 1 rides DVE so the two serial
                        # acc chains run on different engines
                        eng = nc.gpsimd if st == 0 else nc.vector
                        if diag:
                            for half in range(2):
                                q = qo[half]
                                eng.tensor_tensor(et2[:, half, q:q + 128],
                                                  et2[:, half, q:q + 128],
                                                  tril_sb[:], MULT)
                        if u == 0:
                            eng.tensor_copy(acc[st][:], et2[:, 0])
                        else:
                            eng.tensor_tensor(acc[st][:, qo[0]:],
                                              acc[st][:, qo[0]:],
                                              et2[:, 0, qo[0]:], ADD)
                        eng.tensor_tensor(acc[st][:, qo[1]:],
                                          acc[st][:, qo[1]:],
                                          et2[:, 1, qo[1]:], ADD)
                    pending.append((u, et2s, qo))
                    if u >= LAG:
                        emit_pa(*pending.pop(0))
                    if u == 1 and carry_in is not None:
                        carry_in[1]()
                    if u >= 1 and fillers:
                        # previous chunk's output-projection units ride in
                        # the exp-wait bubbles of this pair's pipeline
                        nf = max(1, (len(fillers) + U - u - 1) // (U - u))
                        for _ in range(min(nf, len(fillers))):
                            fillers.pop(0)()
                for item in pending:
                    emit_pa(*item)
                for fl in fillers:
                    fl()
                # partition-reduce the SBUF denominator accumulators: one
                # ones-matmul per stream, into one wide score-pool slot
                ps2f = score_ps.tile([128, 2, QC], F32, tag="ps2",
                                     name=f"ps2f{c}_{hA}")
                for st in range(2):
                    nc.tensor.matmul(ps2f[:, st], ones_sb[:], acc[st][:],
                                     start=True, stop=True)
                # stage the attention accumulator out of PSUM (one wide copy)
                ar2 = araw_pool.tile([128, 2, QC], F32, tag="araw",
                                     name=f"ar2_{c}_{hA}")
                nc.vector.tensor_copy(ar2[:], pa2[:])

                # 1/Z = exp(-ln(Z)): Ln and Exp share an ACT function table,
                # so no ACT_TABLE_LOAD ever splits the exp stream.  fin_a
                # (Ln, reading the PSUM slot directly) runs at the next
                # pair's start; fin_b at its second unit.
                state = {}

                def fin_a():
                    lnt = rc_pool.tile([128, 2, QC], F32, tag="lnt")
                    state["lnt"] = lnt
                    nc.scalar.activation(lnt[:], ps2f[:],
                                         mybir.ActivationFunctionType.Ln)

                def fin_b():
                    rc2 = rc_pool.tile([128, 2, QC], F32, tag="rc4")
                    nc.scalar.activation(rc2[:], state["lnt"][:], EXP,
                                         scale=-1.0)
                    at2 = attn_sb.tile([128, 2, QC], BF16, tag="attnT")
                    nc.vector.tensor_tensor(at2[:], ar2[:], rc2[:], MULT)
                    for st in range(2):
                        attn_tiles[streams[st]] = (at2, st)

                def fin_tail():
                    fin_a()
                    fin_b()
                return fin_a, fin_b, fin_tail

            carry = None
            for c in range(NQC):
                attn_tiles = {}
                carry = emit_pair(c, 0, 1, attn_tiles, carry)
                units = []
                if prev is not None:
                    pc_, pt_ = prev
                    units = [
                        (lambda pu=pu: emit_out_unit(pc_, pt_, pu, fill_ps,
                                                     "fill", nc.vector,
                                                     nc.sync))
                        for pu in range(NKO // 2)
                    ]
                carry = emit_pair(c, 2, 3, attn_tiles, carry, fillers=units)
                prev = (c, attn_tiles)
            carry[2]()
            out_proj_tail(prev[0], prev[1])

    _split_multi_waits(nc)
    return nc


def _get_state():
    if "nc" not in _CACHE:
        _CACHE["nc"] = _build_nc()
        _CACHE["consts"] = _host_consts()
    return _CACHE["nc"], _CACHE["consts"]


def kernel(data=None, mask=None, wq=None, wk=None, wv=None, wo=None, **extra):
    global LAST_RESULTS
    import ml_dtypes
    bf16 = ml_dtypes.bfloat16
    nc, consts = _get_state()

    data = np.asarray(data, dtype=np.float32)
    wq = np.asarray(wq, dtype=np.float32)
    wk = np.asarray(wk, dtype=np.float32)
    wv = np.asarray(wv, dtype=np.float32)
    wo = np.asarray(wo, dtype=np.float32)

    in_maps = []
    # dataT host layout [128, chunk, ko, t]: every DMA element is >=1KB and
    # per-(partition, chunk) spans are 16KB contiguous
    dTs = [np.ascontiguousarray(
        data[b].T.reshape(NKO, 128, NPC, PC).transpose(1, 2, 0, 3)
    ).astype(bf16) for b in range(B)]
    wq_h = [np.ascontiguousarray(
        wq[:, g * GQ:(g + 1) * GQ].reshape(NKO, 128, GQ).transpose(1, 0, 2)
    ).astype(bf16) for g in range(NKV)]
    wkv_h = [np.ascontiguousarray(
        np.concatenate([wk[:, g * HD:(g + 1) * HD],
                        wv[:, g * HD:(g + 1) * HD]], axis=1)
        .reshape(NKO, 128, 2 * HD).transpose(1, 0, 2)
    ).astype(bf16) for g in range(NKV)]
    for b in range(B):
        for g in range(NKV):
            in_maps.append({
                "dataT": dTs[b],
                "wq": wq_h[g],
                "wkv": wkv_h[g],
                "wo": np.ascontiguousarray(wo[g * GQ:(g + 1) * GQ, :]).astype(bf16),
                "cosT": consts["cosT"],
                "sinT": consts["sinT"],
                "ctd": consts["ctd"],
                "sgn": consts["sgn"],
                "rot": consts["rot"],
                "tril": consts["tril"],
                "ones": consts["ones"],
                "ident": consts["ident"],
            })

    res = run_bass_kernel_spmd(nc, in_maps, core_ids=list(range(8)))
    LAST_RESULTS = res

    out = np.empty((B, S, D), dtype=np.float32)
    for b in range(B):
        acc = res.results[b * NKV]["outT"].astype(np.float32).copy()
        for g in range(1, NKV):
            acc += res.results[b * NKV + g]["outT"]
        out[b] = acc.T
    return out



# revision 27
# speedup vs baseline: 1.0451x; 1.0451x over previous
"""Trainium2 Bass kernel for nn_Attention_197568495719.

Full attention layer: QKV projection + RoPE + int8 KV quant-dequant + GQA
causal SDPA + output projection.  B=2, S=2048, D=2048, 16 q heads / 4 kv
heads, head_dim=128.

Sharding: 8 cores = 2 (batch) x 4 (kv-head groups).  Core (b, g) computes
batch b with q heads 4g..4g+3 and kv head g (tensor parallel on heads:
wq/wk/wv split on output dim, wo on input dim).  Each core produces a
partial outT = (attn @ wo_g).T in [D, S] layout; the host sums the 4
group partials per batch and transposes back.

Design (measured ~290us full-clock vs the 485us v1 baseline; the
device times bimodally, ~1.18x slower when power-throttled to ~2GHz):
- Phase 1 (projections): 512-token chunks at N=512 moving; data/wq/wkv
  fed as bf16 (same PE rate as fp32r, half the DMA bytes + SBUF).  k/v
  are projected directly into [token, dim] tiles so the int8 quant path
  needs no PE transposes in; k RoPE runs along the free axis with a
  sign-folded sin table.  All quant-dequant work rides inside phase 1
  (per 512-token group as soon as its k is roped) so the attention phase
  has no serial DVE chains.  q RoPE is applied in place per (head,
  chunk); rot matmuls are deferred behind all four head projections so
  their input copies are long done when the PE reaches them.  A scratch
  fp32 matmul burst warms the PE HAM clock-gate during the initial DMAs.
- Phase 2 (attention): two head-streams interleaved per chunk with
  scores emitted four ki ahead of the accumulating matmuls, so the PE
  always has independent work while ACT exps complete.  Causal masking
  multiplies only the 128x128 triangular block per diagonal tile (on the
  otherwise-idle GPSIMD); exp is trimmed to the live q-range.  Softmax
  accumulators are staged out of PSUM with cheap DVE copies at pair end
  (keeping all 8 PSUM banks rotating; on ACT these copies would queue
  ahead of the next pair's exps and stall score-slot recycling), and 1/sum is computed per pair as
  exp(-ln(sum)) on ACT -- Ln and Exp share an ACT function table, so no
  ACT_TABLE_LOAD ever splits the exp stream (Reciprocal lives in a
  different table; each switch costs 1.28us of serial ACT time).  Each
  pair's finalize is deferred into the middle of the next pair.  The
  previous chunk's output-projection units are interleaved into pair-2's
  ki loop as PE filler; the final output projection is software-pipelined
  7 deep across all 8 PSUM banks (attention accumulators are dead by
  then) with a per-stream split of the last softmax finalize, and its
  store DMAs
  fan out across all three DMA-capable queues.  wo/attn tiles and the
  outT partials are bf16, halving the weight-load and output-store HBM
  traffic shared by all 8 cores (host accumulates partials in fp32).
- All PE matmuls are float32r (full-rate fp32 at moving dim >= 256)
  except the bf16 projections; rounding in the quant path uses the fp32
  +-1.5*2^23 magic-add trick (exact round-half-to-even, matching
  jnp.round).
"""

import numpy as np

import bass_rust
import concourse.bass as bass
import concourse.tile as tile
import concourse.mybir as mybir
from concourse.bass_utils import run_bass_kernel_spmd

B, S, D = 2, 2048, 2048
NH, NKV, HD = 16, 4, 128
GQ = 512            # q dims per core (4 heads)
NKO = D // 128      # 16 contraction tiles
PC = 512            # projection/attention chunk width (tokens)
NPC = S // PC       # 4
QC = 512
NQC = S // QC       # 4
MAGIC = float(np.float32(12582912.0))  # 1.5 * 2**23
SM_SCALE = 1.0 / float(np.sqrt(HD))

F32 = mybir.dt.float32
F32R = mybir.dt.float32r
BF16 = mybir.dt.bfloat16
MULT = mybir.AluOpType.mult
ADD = mybir.AluOpType.add
EXP = mybir.ActivationFunctionType.Exp

_CACHE = {}

# retained after each kernel() call so test harnesses can read profiling info
LAST_RESULTS = None


def _split_multi_waits(nc):
    """This walrus build caps sync waits at 1 per instruction.  Hoist extra
    waits onto single-wait NoOps immediately preceding the instruction on
    the same engine (identical semantics: the engine is in-order)."""
    for f in nc.m.functions:
        for bb in f.blocks:
            new = []
            for inst in bb.instructions:
                si = inst.sync_info
                if si is None:
                    new.append(inst)
                    continue
                waits = list(si.on_wait)
                if len(waits) > 1:
                    for k, w in enumerate(waits[:-1]):
                        nop = mybir.InstNoOp(name=f"{inst.name}-w{k}", ins=[], outs=[])
                        nop.engine = inst.engine
                        nop.sync_info = bass_rust.SyncInfo(on_wait=[w], on_update=[])
                        new.append(nop)
                    inst.sync_info = bass_rust.SyncInfo(
                        on_wait=[waits[-1]], on_update=list(si.on_update)
                    )
                new.append(inst)
            bb.instructions = new


def _host_consts():
    theta = 10000.0
    angles = 1.0 / theta ** (np.arange(0, HD, 2, dtype=np.float32) / HD)
    emb = np.outer(np.arange(S, dtype=np.float32), angles)
    emb = np.concatenate([emb, emb], axis=-1)          # [S, HD]
    cos = np.cos(emb).astype(np.float32)               # [S, HD]
    sin = np.sin(emb).astype(np.float32)
    cosT = np.ascontiguousarray(cos.T)                 # [128, S]
    sinT = np.ascontiguousarray(sin.T)

    # [t, d]-layout tables for k rope: [p, t_tile, hd]
    ctd = np.ascontiguousarray(cos.reshape(S // 128, 128, HD).transpose(1, 0, 2))
    std = sin.reshape(S // 128, 128, HD).transpose(1, 0, 2).copy()
    sgn = std.copy()
    sgn[:, :, : HD // 2] = -std[:, :, : HD // 2]       # sign-folded sin
    sgn = np.ascontiguousarray(sgn)

    rot = np.zeros((128, 128), dtype=np.float32)       # lhsT of rotate_half
    for i in range(64):
        rot[i, i + 64] = 1.0
        rot[i + 64, i] = -1.0

    p = np.arange(128)[:, None]
    f = np.arange(128)[None, :]
    tril = (p <= f).astype(np.float32)                 # key p visible to q f

    ones = np.ones((128, 128), dtype=np.float32)
    ident = np.eye(128, dtype=np.float32)
    import ml_dtypes
    bf16 = ml_dtypes.bfloat16
    return {
        "cosT": cosT.astype(bf16), "sinT": sinT.astype(bf16),
        "ctd": ctd.astype(bf16), "sgn": sgn.astype(bf16),
        "rot": rot.astype(bf16), "tril": tril.astype(bf16),
        "ones": ones.astype(bf16), "ident": ident,
    }


def _build_nc():
    nc = bass.Bass("TRN2", target_bir_lowering=False, debug=False)

    # host pre-arranges dataT/wq/wkv into partition-major layouts so every
    # DMA element is >=4KB contiguous (512B elements run ~3x slower)
    dataT = nc.dram_tensor("dataT", [128, NPC, NKO, PC], BF16,
                           kind="ExternalInput").ap()
    wq = nc.dram_tensor("wq", [128, NKO, GQ], BF16, kind="ExternalInput").ap()
    wkv = nc.dram_tensor("wkv", [128, NKO, 2 * HD], BF16,
                         kind="ExternalInput").ap()
    wo = nc.dram_tensor("wo", [GQ, D], BF16, kind="ExternalInput").ap()
    cosT_d = nc.dram_tensor("cosT", [128, S], BF16, kind="ExternalInput").ap()
    sinT_d = nc.dram_tensor("sinT", [128, S], BF16, kind="ExternalInput").ap()
    ctd_d = nc.dram_tensor("ctd", [128, NKO, HD], BF16, kind="ExternalInput").ap()
    sgn_d = nc.dram_tensor("sgn", [128, NKO, HD], BF16, kind="ExternalInput").ap()
    rot_d = nc.dram_tensor("rot", [128, 128], BF16, kind="ExternalInput").ap()
    tril_d = nc.dram_tensor("tril", [128, 128], BF16, kind="ExternalInput").ap()
    ones_d = nc.dram_tensor("ones", [128, 128], BF16, kind="ExternalInput").ap()
    ident_d = nc.dram_tensor("ident", [128, 128], F32R, kind="ExternalInput").ap()
    outT = nc.dram_tensor("outT", [D, S], BF16, kind="ExternalOutput").ap()

    dataT_r = dataT                                          # [128, 4, 16, PC]
    wq_r = wq                                                # [128, 16, 512]
    wkv_r = wkv                                              # [128, 16, 256]
    wo_r = wo.rearrange("(h p) n -> p h n", p=128)           # [128, 4, S]
    outT_p = outT.rearrange("(dt p) t -> p dt t", p=128)     # [128, 16, S]

    from contextlib import ExitStack
    with tile.TileContext(nc) as tc, ExitStack() as stack:
        small_consts = stack.enter_context(tc.tile_pool(name="sconsts", bufs=1))
        rot_sb = small_consts.tile([128, 128], BF16)
        ones_sb = small_consts.tile([128, 128], BF16)
        id_sb = small_consts.tile([128, 128], F32R)
        tril_sb = small_consts.tile([128, 128], BF16)

        persist = stack.enter_context(tc.tile_pool(name="persist", bufs=1))
        xq4 = persist.tile([128, 4, S], BF16, name="xq4")    # roped q, [d, h, t]
        kt4 = persist.tile([128, 4, QC], BF16, name="kt4")   # quant k, [d, g, t]
        v_g = [persist.tile([128, 4, HD], BF16, tag=f"vg{g}", name=f"v_g{g}")
               for g in range(4)]                            # quant v, [t, j, d]

        GRP = 4

        # ---------------- Phase 1: projections + rope + quant ----------------
        with tc.tile_pool(name="p1consts", bufs=1) as p1c, \
             tc.tile_pool(name="wpool", bufs=1) as wpool, \
             tc.tile_pool(name="datapool", bufs=2) as datapool, \
             tc.tile_pool(name="kvstage", bufs=2) as kvstage, \
             tc.tile_pool(name="qtmp", bufs=2) as qtmp, \
             tc.tile_pool(name="t2pool", bufs=3) as t2pool, \
             tc.tile_pool(name="proj_ps", bufs=3, space="PSUM") as proj_ps, \
             tc.tile_pool(name="kv_ps", bufs=2, space="PSUM") as kv_ps, \
             tc.tile_pool(name="rope_ps", bufs=2, space="PSUM") as rope_ps, \
             tc.tile_pool(name="tp_ps", bufs=1, space="PSUM") as tp_ps:
            cos_sb = p1c.tile([128, S], BF16)
            sin_sb = p1c.tile([128, S], BF16)
            ctd_sb = p1c.tile([128, NKO, HD], BF16)
            sgn_sb = p1c.tile([128, NKO, HD], BF16)
            wq_sb = wpool.tile([128, NKO, GQ], BF16)
            wkv_sb = wpool.tile([128, NKO, 2 * HD], BF16)

            dT = {}
            for c in range(2):
                dT[c] = datapool.tile([128, NKO, PC], BF16, tag="dT",
                                      name=f"dT{c}")

            # initial loads across all 3 DMA-capable queues (sync/gpsimd/
            # scalar).  The DMA arbiter round-robins PACKETS across queues,
            # so a queue carrying small-element descriptors gets starved:
            # big 4-8KB-element transfers go first in each queue's FIFO,
            # ordered by first use (kv0: wkv+dT0, q0: wq, kv1: dT1), and the
            # small consts ride at the queue tails.
            nc.sync.dma_start(dT[0][:, 0:8], dataT_r[:, 0, 0:8])
            nc.gpsimd.dma_start(dT[0][:, 8:16], dataT_r[:, 0, 8:16])
            nc.scalar.dma_start(wkv_sb[:, 0:8], wkv_r[:, 0:8])
            nc.scalar.dma_start(wkv_sb[:, 8:16], wkv_r[:, 8:16])
            nc.sync.dma_start(wq_sb[:, 0:8], wq_r[:, 0:8])
            nc.scalar.dma_start(wq_sb[:, 8:16], wq_r[:, 8:16])
            nc.gpsimd.dma_start(ctd_sb[:], ctd_d[:])
            nc.gpsimd.dma_start(cos_sb[:], cosT_d[:])
            nc.sync.dma_start(dT[1][:, 0:8], dataT_r[:, 1, 0:8])
            nc.gpsimd.dma_start(dT[1][:, 8:16], dataT_r[:, 1, 8:16])
            nc.scalar.dma_start(sgn_sb[:], sgn_d[:])
            nc.scalar.dma_start(sin_sb[:], sinT_d[:])
            nc.sync.dma_start(rot_sb[:], rot_d[:])
            nc.sync.dma_start(id_sb[:], ident_d[:])
            nc.sync.dma_start(ones_sb[:], ones_d[:])
            nc.sync.dma_start(tril_sb[:], tril_d[:])

            # PE warm-up/filler: scratch matmuls during the initial DMA wait
            # keep the HAM clock-gate at K=8/8 so real work runs at full
            # clock, and bridge to the kv projection's first data (~17us in)
            # so no >3.4us idle window re-throttles the clock.
            warm = wpool.tile([128, QC], BF16, name="warm_scratch")
            nc.vector.memset(warm[:], 0.0)

            def warm_fill(n, w=QC):
                for _ in range(n):
                    wps = rope_ps.tile([128, QC], F32, tag="pr")
                    nc.tensor.matmul(wps[:, 0:w], warm[:, 0:128], warm[:, 0:w],
                                     start=True, stop=True)

            warm_fill(22)

            def quant_group(src_ap, dst_ap):
                amax = qtmp.tile([128, GRP, 1], F32, tag="amax")
                scl = qtmp.tile([128, GRP, 1], F32, tag="scl")
                inv = qtmp.tile([128, GRP, 1], F32, tag="inv")
                xs = qtmp.tile([128, GRP, HD], F32, tag="xs")
                nc.vector.tensor_reduce(amax[:], src_ap, mybir.AxisListType.X,
                                        mybir.AluOpType.max,
                                        apply_absolute_value=True)
                nc.vector.tensor_scalar_max(amax[:], amax[:], 1e-8)
                nc.vector.tensor_scalar_mul(scl[:], amax[:], 1.0 / 127.0)
                nc.vector.reciprocal(inv[:], scl[:])
                sclb = scl[:].to_broadcast((128, GRP, HD))
                invb = inv[:].to_broadcast((128, GRP, HD))
                nc.vector.tensor_tensor(xs[:], src_ap, invb, MULT)
                nc.vector.tensor_scalar_add(xs[:], xs[:], MAGIC)
                nc.vector.tensor_scalar_add(xs[:], xs[:], -MAGIC)
                nc.vector.tensor_tensor(dst_ap, xs[:], sclb, MULT)

            for c in range(NPC):
                csl = bass.ts(c, PC)
                if c + 2 < NPC:
                    cb = c + 2
                    t_ = datapool.tile([128, NKO, PC], BF16, tag="dT",
                                       name=f"dT{cb}")
                    dT[cb] = t_
                    eng = nc.gpsimd if cb % 2 else nc.sync
                    eng.dma_start(t_[:, 0:8], dataT_r[:, cb, 0:8])
                    eng.dma_start(t_[:, 8:16], dataT_r[:, cb, 8:16])

                # --- k/v projection straight into [t, d] tiles ---
                kv_td = kvstage.tile([128, GRP, 2 * HD], F32, tag="kvtd",
                                     name=f"kvtd{c}")
                for j in range(GRP):
                    pkv = kv_ps.tile([128, 2 * HD], F32, tag="pkv")
                    for ko in range(NKO):
                        nc.tensor.matmul(pkv[:],
                                         dT[c][:, ko, bass.ds(j * 128, 128)],
                                         wkv_sb[:, ko],
                                         start=(ko == 0), stop=(ko == NKO - 1))
                    nc.scalar.copy(kv_td[:, j, :], pkv[:])

                # --- k rope along free axis (sign-folded sin table) ---
                kr = kvstage.tile([128, GRP, HD], F32, tag="kr", name=f"kr{c}")
                t2k = qtmp.tile([128, GRP, HD], F32, tag="t2k")
                tsl = bass.ts(c, GRP)  # 4 token tiles of this group
                nc.vector.tensor_tensor(kr[:], kv_td[:, :, 0:HD],
                                        ctd_sb[:, tsl], MULT)
                nc.vector.tensor_tensor(t2k[:, :, 0:64],
                                        kv_td[:, :, 64:HD],
                                        sgn_sb[:, tsl, 0:64], MULT)
                nc.vector.tensor_tensor(t2k[:, :, 64:HD],
                                        kv_td[:, :, 0:64],
                                        sgn_sb[:, tsl, 64:HD], MULT)
                nc.vector.tensor_tensor(kr[:], kr[:], t2k[:], ADD)

                # --- int8 quant-dequant (k roped, v raw) ---
                kq = kvstage.tile([128, GRP, HD], F32R, tag="kq", name=f"kq{c}")
                quant_group(kr[:], kq[:])

                # --- q projection per head; kq transposes slotted after
                # h1 so the DVE quant chain has time to finish; rot matmuls
                # deferred behind all projections so their input copies are
                # long done when the PE reaches them ---
                def emit_qproj(h):
                    pq = proj_ps.tile([128, QC], F32, tag="pq",
                                      name=f"pq{c}_{h}")
                    for ko in range(NKO):
                        nc.tensor.matmul(pq[:], wq_sb[:, ko, bass.ts(h, 128)],
                                         dT[c][:, ko],
                                         start=(ko == 0), stop=(ko == NKO - 1))
                    nc.scalar.copy(xq4[:, h, csl], pq[:])

                emit_qproj(0)
                emit_qproj(1)
                for j in range(GRP):
                    pt = tp_ps.tile([128, 128], F32R, tag="tp")
                    nc.tensor.transpose(pt[:], kq[:, j, :], id_sb[:])
                    nc.scalar.copy(kt4[:, c, bass.ts(j, 128)], pt[:])
                emit_qproj(2)
                emit_qproj(3)
                for h in range(4):
                    pr = rope_ps.tile([128, QC], F32, tag="pr")
                    nc.tensor.matmul(pr[:], rot_sb[:], xq4[:, h, csl],
                                     start=True, stop=True)
                    t1 = t2pool.tile([128, QC], BF16, tag="t1")
                    t2 = t2pool.tile([128, QC], BF16, tag="t2")
                    nc.vector.tensor_tensor(t1[:], xq4[:, h, csl],
                                            cos_sb[:, csl], MULT)
                    nc.vector.tensor_tensor(t2[:], pr[:], sin_sb[:, csl], MULT)
                    nc.vector.tensor_tensor(xq4[:, h, csl], t1[:], t2[:], ADD)
                # v_g[c] is not needed until attention chunk c; deferring
                # its DVE chain past the q ropes lets the rope PSUM banks
                # (reused by the next chunk's rot matmuls, and by phase-2
                # accumulator pools after the last chunk) free earlier
                quant_group(kv_td[:, :, HD:], v_g[c][:])

        # ---------------- Phase 2: attention + output projection ----------------
        # ki tiles are processed in units of 2 with [128,2,QC] "wide" tiles
        # spanning 2 PSUM banks / 2KB-per-partition SBUF spans: one exp per
        # off-diagonal unit (amortizes the 352-cycle ACT pipeline fill), one
        # staging copy / store per unit.  The softmax denominator rides the
        # PE as per-ki ones-matmuls (engine-side accumulation measured ~2x
        # slower and starves the PE with serial chains).  Each chunk's
        # output projection runs as a dense PE block at the next chunk
        # boundary, when all 8 PSUM banks are free and the ACT engine gets
        # a breather between exp-heavy pairs.
        with tc.tile_pool(name="p2", bufs=1) as p2, \
             tc.tile_pool(name="attn_sb", bufs=5) as attn_sb, \
             tc.tile_pool(name="exp_pool", bufs=7) as exp_pool, \
             tc.tile_pool(name="araw", bufs=3) as araw_pool, \
             tc.tile_pool(name="rc4p", bufs=2) as rc_pool, \
             tc.tile_pool(name="outstage", bufs=4) as outstage, \
             tc.tile_pool(name="score_ps", bufs=2, space="PSUM") as score_ps, \
             tc.tile_pool(name="attn_ps", bufs=1, space="PSUM") as attn_ps, \
             tc.tile_pool(name="pss_ps", bufs=1, space="PSUM") as pss_ps:
            wo_t = [p2.tile([128, S], BF16, tag=f"wo{h}", name=f"wo{h}")
                    for h in range(4)]
            for h in range(4):
                nc.sync.dma_start(wo_t[h][:], wo_r[:, h])

            def out_proj_block(c_prev, tiles, fin):
                # chunk-boundary block: run the previous pair's softmax
                # finalize, then the whole [D, QC] output projection of
                # chunk c_prev software-pipelined 3 units deep over the 4
                # wide PSUM slots (all free at a chunk boundary); the h0/h1
                # lead covers the finalize chain before h2/h3 need its
                # at-tiles.
                fin()
                pools = [(score_ps, "ps2"), (score_ps, "ps2"),
                         (attn_ps, "pa2"), (pss_ps, "pss2")]
                pos = {}

                def finishp(pu):
                    po2 = pos.pop(pu)
                    for half in range(2):
                        dt_ = 2 * pu + half
                        for h2 in (2, 3):
                            at2, sti = tiles[h2]
                            nc.tensor.matmul(po2[:, half],
                                             wo_t[h2][:, bass.ts(dt_, 128)],
                                             at2[:, sti],
                                             start=False, stop=(h2 == 3))
                    ot2 = outstage.tile([128, 2, QC], BF16, tag="ot")
                    if pu % 2:
                        nc.vector.tensor_copy(ot2[:], po2[:])
                    else:
                        nc.scalar.copy(ot2[:], po2[:])
                    eng = nc.gpsimd if pu % 2 else nc.sync
                    eng.dma_start(outT_p[:, 2 * pu:2 * pu + 2,
                                         bass.ts(c_prev, QC)], ot2[:])

                for pu in range(NKO // 2):
                    pool, tag = pools[pu % 4]
                    po2 = pool.tile([128, 2, QC], F32, tag=tag)
                    pos[pu] = po2
                    for half in range(2):
                        dt_ = 2 * pu + half
                        for h2 in (0, 1):
                            at2, sti = tiles[h2]
                            nc.tensor.matmul(po2[:, half],
                                             wo_t[h2][:, bass.ts(dt_, 128)],
                                             at2[:, sti],
                                             start=(h2 == 0), stop=False)
                    if pu >= 3:
                        finishp(pu - 3)
                for pu in range(NKO // 2 - 3, NKO // 2):
                    finishp(pu)

            LAG = 2  # units the score/exp pipeline leads the pa matmuls by

            def emit_pair(c, hA, hB, attn_tiles, carry_in):
                nki = 4 * (c + 1)
                U = nki // 2
                streams = (hA, hB)
                if carry_in is not None:
                    # previous pair's Ln runs first so its pss slot frees
                    # before this pair's ones-matmuls need it
                    carry_in[0]()
                pa2 = attn_ps.tile([128, 2, QC], F32, tag="pa2",
                                   name=f"pa2_{c}_{hA}")
                pss2 = pss_ps.tile([128, 2, QC], F32, tag="pss2",
                                   name=f"pss2_{c}_{hA}")

                def emit_acc(u, et2s, qo):
                    for st in range(2):
                        for half in range(2):
                            ki = 2 * u + half
                            q = qo[half]
                            nc.tensor.matmul(
                                pss2[:, st, q:], ones_sb[:],
                                et2s[st][:, half, q:],
                                start=(ki == 0), stop=(ki == nki - 1))
                            nc.tensor.matmul(
                                pa2[:, st, q:], v_g[ki // 4][:, ki % 4],
                                et2s[st][:, half, q:],
                                start=(ki == 0), stop=(ki == nki - 1))

                pending = []
                for u in range(U):
                    k0 = 2 * u
                    diag = k0 >= 4 * c
                    qo = (128 * (k0 - 4 * c), 128 * (k0 + 1 - 4 * c)) \
                        if diag else (0, 0)
                    et2s = []
                    for st in range(2):
                        h = streams[st]
                        ps2 = score_ps.tile([128, 2, QC], F32, tag="ps2")
                        for half in range(2):
                            q = qo[half]
                            nc.tensor.matmul(
                                ps2[:, half, q:],
                                kt4[:, (k0 + half) // 4,
                                    bass.ts((k0 + half) % 4, 128)],
                                xq4[:, h, bass.ds(c * QC + q, QC - q)],
                                start=True, stop=True)
                        et2 = exp_pool.tile([128, 2, QC], BF16, tag="et2")
                        et2s.append(et2)
                        if diag:
                            # per-half exps: the trimmed halves start at
                            # different q offsets, and the garbage gap is
                            # unwritten PSUM
                            for half in range(2):
                                q = qo[half]
                                nc.scalar.activation(et2[:, half, q:],
                                                     ps2[:, half, q:], EXP,
                                                     scale=SM_SCALE)
                            for half in range(2):
                                q = qo[half]
                                nc.gpsimd.tensor_tensor(
                                    et2[:, half, q:q + 128],
                                    et2[:, half, q:q + 128],
                                    tril_sb[:], MULT)
                        else:
                            nc.scalar.activation(et2[:], ps2[:], EXP,
                                                 scale=SM_SCALE)
                    pending.append((u, et2s, qo))
                    if u >= LAG:
                        emit_acc(*pending.pop(0))
                    if u == 1 and carry_in is not None:
                        carry_in[1]()
                for item in pending:
                    emit_acc(*item)
                # stage the attention accumulator out of PSUM (one wide copy)
                ar2 = araw_pool.tile([128, 2, QC], F32, tag="araw",
                                     name=f"ar2_{c}_{hA}")
                nc.vector.tensor_copy(ar2[:], pa2[:])

                # 1/Z = exp(-ln(Z)): Ln and Exp share an ACT function table,
                # so no ACT_TABLE_LOAD ever splits the exp stream.  fin_a
                # (Ln, reading the PSUM accumulator directly) runs at the
                # next pair's start; fin_b at its second unit.
                state = {}

                def fin_a():
                    lnt = rc_pool.tile([128, 2, QC], F32, tag="lnt")
                    state["lnt"] = lnt
                    nc.scalar.activation(lnt[:], pss2[:],
                                         mybir.ActivationFunctionType.Ln)

                def fin_b():
                    rc2 = rc_pool.tile([128, 2, QC], F32, tag="rc4")
                    nc.scalar.activation(rc2[:], state["lnt"][:], EXP,
                                         scale=-1.0)
                    at2 = attn_sb.tile([128, 2, QC], BF16, tag="attnT")
                    nc.vector.tensor_tensor(at2[:], ar2[:], rc2[:], MULT)
                    for st in range(2):
                        attn_tiles[streams[st]] = (at2, st)

                def fin_tail():
                    fin_a()
                    fin_b()
                return fin_a, fin_b, fin_tail

            prev = None
            for c in range(NQC):
                attn_tiles = {}
                if prev is not None:
                    out_proj_block(prev[0], prev[1], prev[2])
                carry = emit_pair(c, 0, 1, attn_tiles, None)
                carry = emit_pair(c, 2, 3, attn_tiles, carry)
                prev = (c, attn_tiles, carry[2])
            out_proj_block(prev[0], prev[1], prev[2])

    _split_multi_waits(nc)
    return nc


def _get_state():
    if "nc" not in _CACHE:
        _CACHE["nc"] = _build_nc()
        _CACHE["consts"] = _host_consts()
    return _CACHE["nc"], _CACHE["consts"]


def kernel(data=None, mask=None, wq=None, wk=None, wv=None, wo=None, **extra):
    global LAST_RESULTS
    import ml_dtypes
    bf16 = ml_dtypes.bfloat16
    nc, consts = _get_state()

    data = np.asarray(data, dtype=np.float32)
    wq = np.asarray(wq, dtype=np.float32)
    wk = np.asarray(wk, dtype=np.float32)
    wv = np.asarray(wv, dtype=np.float32)
    wo = np.asarray(wo, dtype=np.float32)

    in_maps = []
    # dataT host layout [128, chunk, ko, t]: every DMA element is >=1KB and
    # per-(partition, chunk) spans are 16KB contiguous
    dTs = [np.ascontiguousarray(
        data[b].T.reshape(NKO, 128, NPC, PC).transpose(1, 2, 0, 3)
    ).astype(bf16) for b in range(B)]
    wq_h = [np.ascontiguousarray(
        wq[:, g * GQ:(g + 1) * GQ].reshape(NKO, 128, GQ).transpose(1, 0, 2)
    ).astype(bf16) for g in range(NKV)]
    wkv_h = [np.ascontiguousarray(
        np.concatenate([wk[:, g * HD:(g + 1) * HD],
                        wv[:, g * HD:(g + 1) * HD]], axis=1)
        .reshape(NKO, 128, 2 * HD).transpose(1, 0, 2)
    ).astype(bf16) for g in range(NKV)]
    for b in range(B):
        for g in range(NKV):
            in_maps.append({
                "dataT": dTs[b],
                "wq": wq_h[g],
                "wkv": wkv_h[g],
                "wo": np.ascontiguousarray(wo[g * GQ:(g + 1) * GQ, :]).astype(bf16),
                "cosT": consts["cosT"],
                "sinT": consts["sinT"],
                "ctd": consts["ctd"],
                "sgn": consts["sgn"],
                "rot": consts["rot"],
                "tril": consts["tril"],
                "ones": consts["ones"],
                "ident": consts["ident"],
            })

    res = run_bass_kernel_spmd(nc, in_maps, core_ids=list(range(8)))
    LAST_RESULTS = res

    out = np.empty((B, S, D), dtype=np.float32)
    for b in range(B):
        acc = res.results[b * NKV]["outT"].astype(np.float32).copy()
        for g in range(1, NKV):
            acc += res.results[b * NKV + g]["outT"]
        out[b] = acc.T
    return out

